# Initial kernel scaffold
#
"""Trainium2 Bass kernel for nn_AdjLeafGNN (encoder + kNN graph + 2-layer GCN).

Self-contained: hardcodes all shapes. Shards the batch of 1024 images over
8 NeuronCores (128 images/core), computes the CNN encoder data-parallel,
AllGathers embeddings, then computes distance/adjacency rows + GCN row-sharded.

Returns (emb, adj, dist, logits_cls, logits_spread) like the reference.
"""
import numpy as np

import concourse.bacc as bacc
import concourse.mybir as mybir
import concourse.tile as tile
from concourse.bass_utils import run_bass_kernel_spmd

dt = mybir.dt
AF = mybir.ActivationFunctionType
ALU = mybir.AluOpType

N_CORES = 8
N = 1024                    # batch / graph nodes
NL = N // N_CORES           # nodes per core = 128
G = 32                      # images per group
NGROUPS = NL // G           # 4
HID = 256
K_NN = 5
EPS_BIAS = 6e-11            # index tie-break bias (d2 units)

# ---------------------------------------------------------------------------
# device program
# ---------------------------------------------------------------------------


def build_nc():
    nc = bacc.Bacc("TRN2", target_bir_lowering=False, num_devices=N_CORES)

    # ---- inputs (per core) ----
    im2col1 = nc.dram_tensor("im2col1", [27, NL * 256], dt.float32, kind="ExternalInput")
    w1c = nc.dram_tensor("w1c", [27, 64], dt.float32, kind="ExternalInput")
    b1c = nc.dram_tensor("b1c", [64, 1], dt.float32, kind="ExternalInput")
    w2t = nc.dram_tensor("w2t", [9, 64, 128], dt.float32, kind="ExternalInput")
    b2c = nc.dram_tensor("b2c", [128, 1], dt.float32, kind="ExternalInput")
    awt = nc.dram_tensor("awt", [4, 9, 128, 128], dt.float32, kind="ExternalInput")
    abt = nc.dram_tensor("abt", [4, 128, 1], dt.float32, kind="ExternalInput")
    pwt = nc.dram_tensor("pwt", [512, 128], dt.float32, kind="ExternalInput")
    pbt = nc.dram_tensor("pbt", [128, 1], dt.float32, kind="ExternalInput")
    fcw = nc.dram_tensor("fcw", [128, 256], dt.float32, kind="ExternalInput")
    fcb = nc.dram_tensor("fcb", [256, 1], dt.float32, kind="ExternalInput")
    gw1 = nc.dram_tensor("gw1", [256, 256], dt.float32, kind="ExternalInput")
    gb1 = nc.dram_tensor("gb1", [256, 1], dt.float32, kind="ExternalInput")
    gw2 = nc.dram_tensor("gw2", [256, 256], dt.float32, kind="ExternalInput")
    gb2 = nc.dram_tensor("gb2", [256, 1], dt.float32, kind="ExternalInput")
    cwt = nc.dram_tensor("cwt", [256, 11], dt.float32, kind="ExternalInput")
    cbt = nc.dram_tensor("cbt", [1, 11], dt.float32, kind="ExternalInput")
    biasrow = nc.dram_tensor("biasrow", [1, N], dt.float32, kind="ExternalInput")
    ident = nc.dram_tensor("ident", [128, 128], dt.float32, kind="ExternalInput")
    # which 128-row block of the global graph this core owns (one-hot-ish):
    # rank r encoded as column offset r*128 handled host-side via per-core biasrow?
    # -> biasrow is global [1, N]; local row index handled by rankoff input
    rankoff = nc.dram_tensor("rankoff", [1, 1], dt.uint32, kind="ExternalInput")

    # ---- outputs (per core) ----
    emb_out = nc.dram_tensor("emb_out", [NL, 256], dt.float32, kind="ExternalOutput")
    dist_out = nc.dram_tensor("dist_out", [NL, N], dt.float32, kind="ExternalOutput")
    adj_out = nc.dram_tensor("adj_out", [NL, N], dt.float32, kind="ExternalOutput")
    cls_out = nc.dram_tensor("cls_out", [NL, 10], dt.float32, kind="ExternalOutput")
    spr_out = nc.dram_tensor("spr_out", [NL, 1], dt.float32, kind="ExternalOutput")

    with tile.TileContext(nc) as tc:
        _body(nc, tc, locals())
    nc.compile()
    return nc


def _body(nc, tc, t):
    ctx_pools = {}

    # ---------------- persistent pools ----------------
    with (
        tc.tile_pool(name="weights", bufs=1) as wp,
        tc.tile_pool(name="persist", bufs=1) as pp,
        tc.tile_pool(name="conv", bufs=1) as cp,
        tc.tile_pool(name="colpool", bufs=2) as colp,
        tc.tile_pool(name="evac", bufs=3) as ep,
        tc.tile_pool(name="psA", bufs=4, space="PSUM") as psA,
        tc.tile_pool(name="psB", bufs=2, space="PSUM") as psB,
        tc.tile_pool(name="dram", bufs=1, space="DRAM") as dramp,
    ):
        # ---------------- load weights ----------------
        w1_sb = wp.tile([27, 64], dt.float32)
        nc.sync.dma_start(w1_sb[:], t["w1c"][:])
        b1_sb = wp.tile([64, 1], dt.float32)
        nc.sync.dma_start(b1_sb[:], t["b1c"][:])
        w2_sb = [wp.tile([64, 128], dt.float32, name=f"w2_sb{i}") for i in range(9)]
        for i in range(9):
            nc.sync.dma_start(w2_sb[i][:], t["w2t"][i])
        b2_sb = wp.tile([128, 1], dt.float32)
        nc.sync.dma_start(b2_sb[:], t["b2c"][:])
        # aspp taps: branch 0 (d=1) and 1 (d=6): all 9; branches 2,3: center only
        aw_sb = {}
        for b in range(4):
            taps = range(9) if b < 2 else [4]
            for tap in taps:
                tl = wp.tile([128, 128], dt.float32, name=f"aw_sb{b}_{tap}")
                nc.sync.dma_start(tl[:], t["awt"][b, tap])
                aw_sb[(b, tap)] = tl
        ab_sb = [wp.tile([128, 1], dt.float32, name=f"ab_sb{b}") for b in range(4)]
        for b in range(4):
            nc.sync.dma_start(ab_sb[b][:], t["abt"][b])
        pw_sb = [wp.tile([128, 128], dt.float32, name=f"pw_sb{b}") for b in range(4)]
        for b in range(4):
            nc.sync.dma_start(pw_sb[b][:], t["pwt"][b * 128:(b + 1) * 128, :])
        pb_sb = wp.tile([128, 1], dt.float32)
        nc.sync.dma_start(pb_sb[:], t["pbt"][:])
        fcw_sb = wp.tile([128, 256], dt.float32)
        nc.sync.dma_start(fcw_sb[:], t["fcw"][:])
        fcb_sb = wp.tile([128, 2], dt.float32)
        nc.sync.dma_start(fcb_sb[:], t["fcb"].rearrange("(m p) o -> p (m o)", p=128))
        ident_sb = wp.tile([128, 128], dt.float32)
        nc.sync.dma_start(ident_sb[:], t["ident"][:])

        gapT = pp.tile([128, NL], dt.float32)  # [gap_ch, node_local]

        # ================= encoder =================
        for g in range(NGROUPS):
            col = colp.tile([27, G * 256], dt.float32, tag="col")
            nc.sync.dma_start(col[:], t["im2col1"][:, g * G * 256:(g + 1) * G * 256])

            # conv1: 16 chunks of N=512
            h1 = cp.tile([64, G * 256], dt.float32, tag="h1")
            for ch in range(G * 256 // 512):
                ps = psA.tile([64, 512], dt.float32, tag="c1ps")
                nc.tensor.matmul(ps[:], w1_sb[:], col[:, ch * 512:(ch + 1) * 512],
                                 start=True, stop=True)
                nc.scalar.activation(h1[:, ch * 512:(ch + 1) * 512], ps[:],
                                     AF.Relu, bias=b1_sb[:, 0:1])

            # conv2: stride 2, 16x16 -> 8x8, 9 taps, center (1,1) first
            h2 = cp.tile([128, G * 64], dt.float32, tag="h2")
            h1v = h1.rearrange("c (i y x) -> c i y x", i=G, y=16, x=16)
            for ch in range(G // 8):  # chunks of 8 images, N=512
                ps = psA.tile([128, 512], dt.float32, tag="c2ps")
                psv = ps.rearrange("o (i y x) -> o i y x", i=8, y=8, x=8)
                i0 = ch * 8
                taps = [(1, 1)] + [(ky, kx) for ky in range(3) for kx in range(3)
                                   if (ky, kx) != (1, 1)]
                for ti, (ky, kx) in enumerate(taps):
                    y0 = 1 if ky == 0 else 0
                    x0 = 1 if kx == 0 else 0
                    # in coords: iy = 2y+ky-1 for y in [y0,8) -> start 2*y0+ky-1
                    rhs = h1v[:, i0:i0 + 8,
                              2 * y0 + ky - 1:16:2,
                              2 * x0 + kx - 1:16:2]
                    outp = psv[:, :, y0:8, x0:8]
                    nc.tensor.matmul(outp, w2_sb[ky * 3 + kx][:], rhs,
                                     start=(ti == 0), stop=(ti == len(taps) - 1),
                                     skip_group_check=True)
                nc.scalar.activation(h2[:, ch * 512:(ch + 1) * 512], ps[:],
                                     AF.Relu, bias=b2_sb[:, 0:1])

            # aspp branches
            h2v = h2.rearrange("c (i y x) -> c i y x", i=G, y=8, x=8)
            aouts = []
            for b, d in enumerate((1, 6, 12, 18)):
                ao = cp.tile([128, G * 64], dt.float32, tag=f"aspp{b}",
                             name=f"aspp{b}")
                for ch in range(G // 8):
                    ps = psA.tile([128, 512], dt.float32, tag="apps")
                    psv = ps.rearrange("o (i y x) -> o i y x", i=8, y=8, x=8)
                    i0 = ch * 8
                    if b >= 2:
                        taps = [(1, 1)]
                    else:
                        taps = [(1, 1)] + [(ky, kx) for ky in range(3)
                                           for kx in range(3) if (ky, kx) != (1, 1)]
                    for ti, (ky, kx) in enumerate(taps):
                        # valid out y: y + d*(ky-1) in [0,8)
                        y0 = max(0, -d * (ky - 1))
                        y1 = min(8, 8 - d * (ky - 1))
                        x0 = max(0, -d * (kx - 1))
                        x1 = min(8, 8 - d * (kx - 1))
                        rhs = h2v[:, i0:i0 + 8,
                                  y0 + d * (ky - 1):y1 + d * (ky - 1),
                                  x0 + d * (kx - 1):x1 + d * (kx - 1)]
                        outp = psv[:, :, y0:y1, x0:x1]
                        nc.tensor.matmul(outp, aw_sb[(b, ky * 3 + kx)][:], rhs,
                                         start=(ti == 0), stop=(ti == len(taps) - 1),
                                         skip_group_check=True)
                    nc.scalar.activation(ao[:, ch * 512:(ch + 1) * 512], ps[:],
                                         AF.Relu, bias=ab_sb[b][:, 0:1])
                aouts.append(ao)

            # proj 1x1 (K=512 over 4 branch tiles) + relu
            hp = cp.tile([128, G * 64], dt.float32, tag="hp")
            for ch in range(G // 8):
                ps = psA.tile([128, 512], dt.float32, tag="prps")
                for b in range(4):
                    nc.tensor.matmul(ps[:], pw_sb[b][:],
                                     aouts[b][:, ch * 512:(ch + 1) * 512],
                                     start=(b == 0), stop=(b == 3))
                nc.scalar.activation(hp[:, ch * 512:(ch + 1) * 512], ps[:],
                                     AF.Relu, bias=pb_sb[:, 0:1])

            # gap: mean over 64 spatial
            gsum = ep.tile([128, G], dt.float32, tag="gsum")
            nc.vector.tensor_reduce(gsum[:], hp.rearrange("c (i s) -> c i s", i=G, s=64),
                                    axis=mybir.AxisListType.X, op=ALU.add)
            nc.scalar.activation(gapT[:, g * G:(g + 1) * G], gsum[:], AF.Copy,
                                 scale=1.0 / 64.0)

        # ---- fc: embT [256, NL] = fcw.T @ gapT + fcb ----
        embT = [pp.tile([128, NL], dt.float32, name=f"embT{m}") for m in range(2)]
        emb_sb = pp.tile([NL, 256], dt.float32)
        for m in range(2):
            ps = psB.tile([128, NL], dt.float32, tag="fcps")
            nc.tensor.matmul(ps[:], fcw_sb[:, m * 128:(m + 1) * 128], gapT[:],
                             start=True, stop=True)
            nc.scalar.activation(embT[m][:], ps[:], AF.Copy, bias=fcb_sb[:, m:m + 1])
            # emb output (node-major) via PE transpose
            tp = psB.tile([128, 128], dt.float32, tag="trps")
            nc.tensor.transpose(tp[:], embT[m][:], ident_sb[:])
            nc.scalar.activation(emb_sb[:, m * 128:(m + 1) * 128], tp[:], AF.Copy)
        nc.sync.dma_start(t["emb_out"][:], emb_sb[:])

        # ---- AllGather embT ----
        ag_in = dramp.tile([256, NL], dt.float32)
        ag_out = dramp.tile([256 * N_CORES, NL], dt.float32, addr_space="Shared")
        for m in range(2):
            nc.sync.dma_start(ag_in[m * 128:(m + 1) * 128, :], embT[m][:])
        nc.gpsimd.collective_compute(
            "AllGather", ALU.bypass,
            replica_groups=[list(range(N_CORES))],
            ins=[ag_in[:]], outs=[ag_out[:]],
        )

        # ---- load embT_all as [128, 16*128]; blocks b=2r+k ----
        ebT = pp.tile([128, 16 * 128], dt.float32)
        nc.sync.dma_start(ebT[:], ag_out.rearrange("(b p) c -> p b c", p=128))

        # ---- mean over nodes per feature (chunk k: blocks k::2) ----
        ebTv = ebT.rearrange("p (r k c) -> p r k c", r=8, k=2, c=128)
        mean = ep.tile([128, 2], dt.float32, tag="mean")
        msum = ep.tile([128, 2], dt.float32, tag="msum")
        for k in range(2):
            nc.vector.tensor_reduce(msum[:, k:k + 1], ebTv[:, :, k, :],
                                    axis=mybir.AxisListType.XY, op=ALU.add)
        nc.scalar.activation(mean[:], msum[:], AF.Copy, scale=1.0 / float(N))

        # ---- centered blocks + squares + sq row ----
        ebC = pp.tile([128, 16 * 128], dt.float32)
        ebCv = ebC.rearrange("p (r k c) -> p r k c", r=8, k=2, c=128)
        for k in range(2):
            nc.vector.tensor_scalar(ebCv[:, :, k, :], ebTv[:, :, k, :],
                                    mean[:, k:k + 1], None, op0=ALU.subtract)
        esq = ep.tile([128, 16 * 128], dt.float32, tag="esq")
        nc.scalar.activation(esq[:], ebC[:], AF.Square)
        ones_col = wp.tile([128, 1], dt.float32)
        nc.vector.memset(ones_col[:], 1.0)
        sq_ps = psB.tile([1, N], dt.float32, tag="sqps")
        esqv = esq.rearrange("p (r k c) -> p r k c", r=8, k=2, c=128)
        for r in range(8):
            for k in range(2):
                nc.tensor.matmul(sq_ps[:, r * 128:(r + 1) * 128], ones_col[:],
                                 esqv[:, r, k, :], start=(k == 0), stop=(k == 1))
        sq_sb = ep.tile([1, N], dt.float32, tag="sqsb")
        nc.scalar.activation(sq_sb[:], sq_ps[:], AF.Copy)

        # lhsT tiles for d2: [-2*e'T_local; ones; sq_local]
        # local block r = rank: slice via rankoff-indexed DMA is awkward; instead
        # every core's own block is at blocks (2r+k) with r = partition id.
        # We use a host-supplied one-hot: simpler = DMA from ag_out using rankoff?
        # Simplest robust: recompute from own embT tiles (uncentered) - mean.
        lhs_k = [pp.tile([128, 128], dt.float32, name=f"lhs_k{k}") for k in range(2)]
        for k in range(2):
            # -2 * (embT_local - mean) = -2*embT + 2*mean
            nc.vector.tensor_scalar(
                lhs_k[k][:], embT[k][:], mean[:, k:k + 1], None,
                op0=ALU.subtract)
            nc.vector.tensor_scalar(lhs_k[k][:], lhs_k[k][:], -2.0, None,
                                    op0=ALU.mult)
        # sq_local row: need own 128-slice of sq_sb on partition 1 of k2 tile.
        # own slice position = rank*128: use indirect? Instead compute locally:
        # sq_local = sum_k colsum(own centered^2). Build from lhs (= -2 e'):
        # e'^2 = (lhs/2)^2 = lhs^2 * 0.25
        lsq = ep.tile([128, 128], dt.float32, tag="lsq")
        sql_ps = psB.tile([1, 128], dt.float32, tag="sqlps")
        for k in range(2):
            nc.scalar.activation(lsq[:], lhs_k[k][:], AF.Square, scale=0.5)
            # note: Square(x*0.5) = 0.25 x^2 = e'^2  (scale applies BEFORE func)
            nc.tensor.matmul(sql_ps[:], ones_col[:], lsq[:],
                             start=(k == 0), stop=(k == 1))
        k2_lhs = pp.tile([2, 128], dt.float32)
        nc.vector.memset(k2_lhs[0:1, :], 1.0)
        nc.scalar.activation(k2_lhs[1:2, :], sql_ps[:], AF.Copy)

        # rhs k2 tiles: row0 = sq_all, row1 = ones
        k2_rhs = pp.tile([2, N], dt.float32)
        nc.vector.memset(k2_rhs[1:2, :], 1.0)
        nc.vector.tensor_copy(k2_rhs[0:1, :], sq_sb[:])

        ones_row = wp.tile([1, 128], dt.float32)
        nc.vector.memset(ones_row[:], 1.0)
        br_sb = ep.tile([1, N], dt.float32, tag="brsb")
        nc.sync.dma_start(br_sb[:], t["biasrow"][:])

        # ---- d2 + dist + selection ----
        dist_sb = pp.tile([NL, N], dt.float32)
        adj_sb = pp.tile([NL, N], dt.float32)
        negd = pp.tile([NL, N], dt.float32)
        for half in range(2):  # N-chunks of 512 = 4 rank blocks
            ps = psB.tile([128, 512], dt.float32, tag="d2ps")
            cs = slice(half * 512, (half + 1) * 512)
            for k in range(2):
                rhs = ebCv[:, 4 * half:4 * half + 4, k, :]
                nc.tensor.matmul(ps[:], lhs_k[k][:], rhs, start=(k == 0),
                                 stop=False, skip_group_check=True)
            nc.tensor.matmul(ps[:], k2_lhs[:], k2_rhs[:, cs], start=False,
                             stop=True, skip_group_check=True)
            # dist = sqrt(relu(d2))
            rl = ep.tile([128, 512], dt.float32, tag="rl")
            nc.scalar.activation(rl[:], ps[:], AF.Relu)
            nc.scalar.activation(dist_sb[:, cs], rl[:], AF.Sqrt)
            # bias accumulate AFTER the reads above
            nc.tensor.matmul(ps[:], ones_row[:], br_sb[:, cs], start=False,
                             stop=True, skip_group_check=True)
            nc.scalar.activation(negd[:, cs], ps[:], AF.Copy, scale=-1.0)
        nc.sync.dma_start(t["dist_out"][:], dist_sb[:])

        m8 = ep.tile([128, 8], dt.float32, tag="m8")
        nc.vector.max(m8[:], negd[:])
        nc.vector.tensor_scalar(adj_sb[:], negd[:], m8[:, 5:6], None, op0=ALU.is_ge)
        nc.sync.dma_start(t["adj_out"][:], adj_sb[:])

        # ---- adjT blocks via PE transpose ----
        adjT = [pp.tile([128, 128], dt.float32, name=f"adjT{r}") for r in range(8)]
        for r in range(8):
            tp = psB.tile([128, 128], dt.float32, tag="trps")
            nc.tensor.transpose(tp[:], adj_sb[:, r * 128:(r + 1) * 128], ident_sb[:])
            nc.scalar.activation(adjT[r][:], tp[:], AF.Copy)

        # ---- gcn layer 1: h1g[r] = emb_all[r] @ gw1 (node-major) ----
        gw1_sb = [wp.tile([128, 256], dt.float32, name=f"gw1_sb{k}") for k in range(2)]
        for k in range(2):
            nc.sync.dma_start(gw1_sb[k][:], t["gw1"][k * 128:(k + 1) * 128, :])
        gw2_sb = [wp.tile([128, 256], dt.float32, name=f"gw2_sb{k}") for k in range(2)]
        for k in range(2):
            nc.sync.dma_start(gw2_sb[k][:], t["gw2"][k * 128:(k + 1) * 128, :])
        gb1_sb = wp.tile([128, 2], dt.float32)
        nc.sync.dma_start(gb1_sb[:], t["gb1"].rearrange("(m p) o -> p (m o)", p=128))
        gb2_sb = wp.tile([128, 2], dt.float32)
        nc.sync.dma_start(gb2_sb[:], t["gb2"].rearrange("(m p) o -> p (m o)", p=128))

        hg1 = [pp.tile([128, 256], dt.float32, name=f"hg1_{r}") for r in range(8)]
        for r in range(8):
            ps = psB.tile([128, 256], dt.float32, tag="hgps")
            for k in range(2):
                nc.tensor.matmul(ps[:], ebTv[:, r, k, :], gw1_sb[k][:],
                                 start=(k == 0), stop=(k == 1))
            nc.scalar.activation(hg1[r][:], ps[:], AF.Copy)

        # x1T[m] = sum_r hg1[r][:,m].T @ adjT[r]  + relu + b1
        x1T = [pp.tile([128, 128], dt.float32, name=f"x1T{m}") for m in range(2)]
        for m in range(2):
            ps = psB.tile([128, 128], dt.float32, tag="xps")
            for r in range(8):
                nc.tensor.matmul(ps[:], hg1[r][:, m * 128:(m + 1) * 128], adjT[r][:],
                                 start=(r == 0), stop=(r == 7))
            nc.scalar.activation(x1T[m][:], ps[:], AF.Relu, bias=gb1_sb[:, m:m + 1])

        # ---- AllGather x1T ----
        ag2_in = dramp.tile([256, NL], dt.float32)
        ag2_out = dramp.tile([256 * N_CORES, NL], dt.float32, addr_space="Shared")
        for m in range(2):
            nc.sync.dma_start(ag2_in[m * 128:(m + 1) * 128, :], x1T[m][:])
        nc.gpsimd.collective_compute(
            "AllGather", ALU.bypass,
            replica_groups=[list(range(N_CORES))],
            ins=[ag2_in[:]], outs=[ag2_out[:]],
        )
        x1a = pp.tile([128, 16 * 128], dt.float32)
        nc.sync.dma_start(x1a[:], ag2_out.rearrange("(b p) c -> p b c", p=128))
        x1av = x1a.rearrange("p (r k c) -> p r k c", r=8, k=2, c=128)

        # gcn layer 2
        hg2 = [pp.tile([128, 256], dt.float32, name=f"hg2_{r}") for r in range(8)]
        for r in range(8):
            ps = psB.tile([128, 256], dt.float32, tag="hgps")
            for k in range(2):
                nc.tensor.matmul(ps[:], x1av[:, r, k, :], gw2_sb[k][:],
                                 start=(k == 0), stop=(k == 1))
            nc.scalar.activation(hg2[r][:], ps[:], AF.Copy)
        x2T = [pp.tile([128, 128], dt.float32, name=f"x2T{m}") for m in range(2)]
        for m in range(2):
            ps = psB.tile([128, 128], dt.float32, tag="xps")
            for r in range(8):
                nc.tensor.matmul(ps[:], hg2[r][:, m * 128:(m + 1) * 128], adjT[r][:],
                                 start=(r == 0), stop=(r == 7))
            nc.scalar.activation(x2T[m][:], ps[:], AF.Copy, bias=gb2_sb[:, m:m + 1])

        # ---- heads ----
        cw_sb = [wp.tile([128, 11], dt.float32, name=f"cw_sb{k}") for k in range(2)]
        for k in range(2):
            nc.sync.dma_start(cw_sb[k][:], t["cwt"][k * 128:(k + 1) * 128, :])
        cb_sb = wp.tile([1, 11], dt.float32)
        nc.sync.dma_start(cb_sb[:], t["cbt"][:])
        ps = psB.tile([128, 11], dt.float32, tag="hdps")
        for k in range(2):
            nc.tensor.matmul(ps[:], x2T[k][:], cw_sb[k][:], start=(k == 0),
                             stop=False, skip_group_check=True)
        nc.tensor.matmul(ps[:], ones_row[:], cb_sb[:], start=False, stop=True,
                         skip_group_check=True)
        logit_sb = ep.tile([128, 11], dt.float32, tag="logit")
        nc.scalar.activation(logit_sb[:], ps[:], AF.Copy)
        nc.sync.dma_start(t["cls_out"][:], logit_sb[:, 0:10])
        nc.sync.dma_start(t["spr_out"][:], logit_sb[:, 10:11])


# ---------------------------------------------------------------------------
# host side
# ---------------------------------------------------------------------------

_NC_CACHE = {}


def _get_nc():
    if "nc" not in _NC_CACHE:
        _NC_CACHE["nc"] = build_nc()
    return _NC_CACHE["nc"]


def _f32(x):
    return np.ascontiguousarray(x, dtype=np.float32)


def prep_in_maps(images, w1, b1, w2, b2, aspp_w, aspp_b, proj_w, proj_b,
                 fc_w, fc_b, gcn1_w, gcn1_b, gcn2_w, gcn2_b, cls_w, cls_b,
                 spr_w, spr_b):
    # conv1 im2col on host: [27(ky,kx,c), 1024, 256]
    xp = np.pad(_f32(images), ((0, 0), (0, 0), (1, 1), (1, 1)))
    s = xp.strides
    v = np.lib.stride_tricks.as_strided(
        xp, (N, 3, 3, 3, 16, 16), (s[0], s[1], s[2], s[3], 2 * s[2], 2 * s[3]))
    col = _f32(v.transpose(2, 3, 1, 0, 4, 5).reshape(27, N, 256))

    vsr = np.float32(1.0) / np.sqrt(np.float32(6.0))
    c6 = np.float32(vsr * vsr)

    shared = dict(
        w1c=_f32(np.asarray(w1).transpose(2, 3, 1, 0).reshape(27, 64)),
        b1c=_f32(b1).reshape(64, 1),
        w2t=_f32(np.asarray(w2).transpose(2, 3, 1, 0).reshape(9, 64, 128)),
        b2c=_f32(b2).reshape(128, 1),
        awt=_f32(np.asarray(aspp_w).transpose(0, 3, 4, 2, 1).reshape(4, 9, 128, 128)),
        abt=_f32(aspp_b).reshape(4, 128, 1),
        pwt=_f32(np.asarray(proj_w)[:, :, 0, 0].T),
        pbt=_f32(proj_b).reshape(128, 1),
        fcw=_f32(fc_w),
        fcb=_f32(fc_b).reshape(256, 1),
        gw1=_f32(c6 * np.asarray(gcn1_w, dtype=np.float32)),
        gb1=_f32(gcn1_b).reshape(256, 1),
        gw2=_f32(c6 * np.asarray(gcn2_w, dtype=np.float32)),
        gb2=_f32(gcn2_b).reshape(256, 1),
        cwt=_f32(np.concatenate([np.asarray(cls_w),
                                 np.asarray(spr_w)], axis=1)),
        cbt=_f32(np.concatenate([np.asarray(cls_b),
                                 np.asarray(spr_b)]))[None, :],
        biasrow=_f32(np.arange(N) * EPS_BIAS)[None, :],
        ident=np.eye(128, dtype=np.float32),
    )
    in_maps = []
    for c in range(N_CORES):
        m = dict(shared)
        m["im2col1"] = _f32(col[:, c * NL:(c + 1) * NL, :].reshape(27, NL * 256))
        m["rankoff"] = np.array([[c]], dtype=np.uint32)
        in_maps.append(m)
    return in_maps


def kernel(**inputs):
    nc = _get_nc()
    in_maps = prep_in_maps(**inputs)
    res = run_bass_kernel_spmd(nc, in_maps, core_ids=list(range(N_CORES)))
    emb = np.concatenate([res.results[c]["emb_out"] for c in range(N_CORES)], 0)
    adj = np.concatenate([res.results[c]["adj_out"] for c in range(N_CORES)], 0)
    dist = np.concatenate([res.results[c]["dist_out"] for c in range(N_CORES)], 0)
    lc = np.concatenate([res.results[c]["cls_out"] for c in range(N_CORES)], 0)
    ls = np.concatenate([res.results[c]["spr_out"][:, 0] for c in range(N_CORES)], 0)
    return emb, adj, dist, lc, ls


# revision 10
# speedup vs baseline: 1.0407x; 1.0407x over previous
"""Trainium2 Bass kernel for nn_AdjLeafGNN (encoder + kNN graph + 2-layer GCN).

Self-contained: hardcodes all shapes. Shards the batch of 1024 images over
8 NeuronCores (128 images/core), computes the CNN encoder data-parallel,
AllGathers embeddings, then computes distance/adjacency rows + GCN row-sharded.

Returns (emb, adj, dist, logits_cls, logits_spread) like the reference.
"""
import numpy as np

import concourse.bacc as bacc
import concourse.mybir as mybir
import concourse.tile as tile
from concourse.bass_utils import run_bass_kernel_spmd

dt = mybir.dt
AF = mybir.ActivationFunctionType
ALU = mybir.AluOpType

N_CORES = 8
N = 1024                    # batch / graph nodes
NL = N // N_CORES           # nodes per core = 128
G = 32                      # images per group
NGROUPS = NL // G           # 4
EPS_BIAS = 5e-11            # index tie-break bias (d2 units)

# ---------------------------------------------------------------------------
# device program
# ---------------------------------------------------------------------------


def build_nc():
    nc = bacc.Bacc("TRN2", target_bir_lowering=False, num_devices=N_CORES)

    t = {}

    def inp(name, shape):
        t[name] = nc.dram_tensor(name, shape, dt.float32, kind="ExternalInput")

    def outp(name, shape):
        t[name] = nc.dram_tensor(name, shape, dt.float32, kind="ExternalOutput")

    inp("im2col1", [27, NL * 256])
    inp("w1c", [27, 64])
    inp("b1c", [64, 1])
    inp("w2t", [9, 64, 128])
    inp("b2c", [128, 1])
    inp("awt", [4, 9, 128, 128])
    inp("abt", [4, 128, 1])
    inp("pwt", [512, 128])
    inp("pbt", [128, 1])
    inp("fcw", [128, 256])
    inp("fcb", [256, 1])
    inp("gw1", [256, 256])
    inp("gb1", [256, 1])
    inp("gw2", [256, 256])
    inp("gb2", [256, 1])
    inp("cwt", [256, 11])
    inp("cbt", [1, 11])
    inp("biasrow", [1, N])
    inp("ident", [128, 128])

    outp("emb_out", [NL, 256])
    outp("dist_out", [NL, N])
    outp("adj_out", [NL, N])
    outp("cls_out", [NL, 10])
    outp("spr_out", [NL, 1])

    with tile.TileContext(nc) as tc:
        _body(nc, tc, t)
    nc.compile()
    return nc


def _encoder(nc, tc, t, wp, pp, psA, gapT):
    """conv stack -> gapT [128ch, NL nodes]."""
    # ---- weights ----
    w1_sb = wp.tile([27, 64], dt.float32)
    nc.sync.dma_start(w1_sb[:], t["w1c"][:])
    b1_sb = wp.tile([64, 1], dt.float32)
    nc.sync.dma_start(b1_sb[:], t["b1c"][:])
    w2_sb = [wp.tile([64, 128], dt.float32, name=f"w2_sb{i}") for i in range(9)]
    for i in range(9):
        nc.sync.dma_start(w2_sb[i][:], t["w2t"][i])
    b2_sb = wp.tile([128, 1], dt.float32)
    nc.sync.dma_start(b2_sb[:], t["b2c"][:])
    aw_sb = {}
    for b in range(4):
        for tap in (range(9) if b < 2 else [4]):
            tl = wp.tile([128, 128], dt.float32, name=f"aw_sb{b}_{tap}")
            nc.sync.dma_start(tl[:], t["awt"][b, tap])
            aw_sb[(b, tap)] = tl
    ab_sb = [wp.tile([128, 1], dt.float32, name=f"ab_sb{b}") for b in range(4)]
    for b in range(4):
        nc.sync.dma_start(ab_sb[b][:], t["abt"][b])
    pw_sb = [wp.tile([128, 128], dt.float32, name=f"pw_sb{b}") for b in range(4)]
    for b in range(4):
        nc.sync.dma_start(pw_sb[b][:], t["pwt"][b * 128:(b + 1) * 128, :])
    pb_sb = wp.tile([128, 1], dt.float32)
    nc.sync.dma_start(pb_sb[:], t["pbt"][:])

    with (
        tc.tile_pool(name="conv", bufs=1) as cp,
        tc.tile_pool(name="colp", bufs=2) as colp,
        tc.tile_pool(name="enc_evac", bufs=3) as ep,
    ):
        for g in range(NGROUPS):
            # conv1: 16 chunks of N=512; im2col slab loaded in 2 halves
            h1 = cp.tile([64, G * 256], dt.float32, tag="h1")
            for hh in range(2):
                col = colp.tile([27, G * 128], dt.float32, tag="col")
                off = g * G * 256 + hh * G * 128
                nc.sync.dma_start(col[:], t["im2col1"][:, off:off + G * 128])
                for ch in range(G * 128 // 512):
                    ps = psA.tile([128, 512], dt.float32, tag="cps")
                    nc.tensor.matmul(ps[0:64, :], w1_sb[:],
                                     col[:, ch * 512:(ch + 1) * 512],
                                     start=True, stop=True)
                    o = hh * G * 128 + ch * 512
                    nc.scalar.activation(h1[:, o:o + 512], ps[0:64, :],
                                         AF.Relu, bias=b1_sb[:, 0:1])

            # conv2: stride 2, 16x16 -> 8x8, 9 taps, center (1,1) first
            h2 = cp.tile([128, G * 64], dt.float32, tag="h2")
            h1v = h1.rearrange("c (i y x) -> c i y x", i=G, y=16, x=16)
            taps9 = [(1, 1)] + [(ky, kx) for ky in range(3) for kx in range(3)
                                if (ky, kx) != (1, 1)]
            for ch in range(G // 8):  # chunks of 8 images, N=512
                ps = psA.tile([128, 512], dt.float32, tag="cps")
                psv = ps.rearrange("o (i y x) -> o i y x", i=8, y=8, x=8)
                i0 = ch * 8
                for ti, (ky, kx) in enumerate(taps9):
                    y0 = 1 if ky == 0 else 0
                    x0 = 1 if kx == 0 else 0
                    ys, xs = 2 * y0 + ky - 1, 2 * x0 + kx - 1
                    rhs = h1v[:, i0:i0 + 8,
                              ys:ys + 2 * (8 - y0) - 1:2,
                              xs:xs + 2 * (8 - x0) - 1:2]
                    nc.tensor.matmul(psv[:, :, y0:8, x0:8], w2_sb[ky * 3 + kx][:],
                                     rhs, start=(ti == 0), stop=(ti == 8),
                                     skip_group_check=True)
                nc.scalar.activation(h2[:, ch * 512:(ch + 1) * 512], ps[:],
                                     AF.Relu, bias=b2_sb[:, 0:1])

            # aspp branches (d=12,18 reduce to 1x1: only center tap in-bounds)
            h2v = h2.rearrange("c (i y x) -> c i y x", i=G, y=8, x=8)
            aouts = []
            for b, d in enumerate((1, 6, 12, 18)):
                ao = cp.tile([128, G * 64], dt.float32, tag=f"aspp{b}",
                             name=f"aspp{b}")
                taps = taps9 if b < 2 else [(1, 1)]
                for ch in range(G // 8):
                    ps = psA.tile([128, 512], dt.float32, tag="cps")
                    psv = ps.rearrange("o (i y x) -> o i y x", i=8, y=8, x=8)
                    i0 = ch * 8
                    for ti, (ky, kx) in enumerate(taps):
                        y0 = max(0, -d * (ky - 1))
                        y1 = min(8, 8 - d * (ky - 1))
                        x0 = max(0, -d * (kx - 1))
                        x1 = min(8, 8 - d * (kx - 1))
                        rhs = h2v[:, i0:i0 + 8,
                                  y0 + d * (ky - 1):y1 + d * (ky - 1),
                                  x0 + d * (kx - 1):x1 + d * (kx - 1)]
                        nc.tensor.matmul(psv[:, :, y0:y1, x0:x1],
                                         aw_sb[(b, ky * 3 + kx)][:], rhs,
                                         start=(ti == 0),
                                         stop=(ti == len(taps) - 1),
                                         skip_group_check=True)
                    nc.scalar.activation(ao[:, ch * 512:(ch + 1) * 512], ps[:],
                                         AF.Relu, bias=ab_sb[b][:, 0:1])
                aouts.append(ao)

            # proj 1x1 (K=512 over 4 branch tiles) + relu
            hp = cp.tile([128, G * 64], dt.float32, tag="hp")
            for ch in range(G // 8):
                ps = psA.tile([128, 512], dt.float32, tag="cps")
                for b in range(4):
                    nc.tensor.matmul(ps[:], pw_sb[b][:],
                                     aouts[b][:, ch * 512:(ch + 1) * 512],
                                     start=(b == 0), stop=(b == 3))
                nc.scalar.activation(hp[:, ch * 512:(ch + 1) * 512], ps[:],
                                     AF.Relu, bias=pb_sb[:, 0:1])

            # gap: mean over 64 spatial positions
            gsum = ep.tile([128, G], dt.float32, tag="gsum")
            nc.vector.tensor_reduce(gsum[:],
                                    hp.rearrange("c (i s) -> c i s", i=G, s=64),
                                    axis=mybir.AxisListType.X, op=ALU.add)
            nc.scalar.activation(gapT[:, g * G:(g + 1) * G], gsum[:], AF.Copy,
                                 scale=1.0 / 64.0)


def _body(nc, tc, t):
    with (
        tc.tile_pool(name="wp", bufs=1) as wp,
        tc.tile_pool(name="pp", bufs=1) as pp,
        tc.tile_pool(name="psA", bufs=4, space="PSUM") as psA,
        tc.tile_pool(name="psB", bufs=2, space="PSUM") as psB,
        tc.tile_pool(name="dram", bufs=1, space="DRAM") as dramp,
    ):
        ident_sb = wp.tile([128, 128], dt.float32)
        nc.sync.dma_start(ident_sb[:], t["ident"][:])
        ones_col = wp.tile([128, 1], dt.float32)
        nc.vector.memset(ones_col[:], 1.0)
        ones_row = wp.tile([1, 128], dt.float32)
        nc.vector.memset(ones_row[:], 1.0)

        gapT = pp.tile([128, NL], dt.float32)
        _encoder(nc, tc, t, wp, pp, psA, gapT)

        # ---- fc: embT [256, NL] = fcw.T @ gapT + fcb; emb output ----
        fcw_sb = wp.tile([128, 256], dt.float32)
        nc.sync.dma_start(fcw_sb[:], t["fcw"][:])
        fcb_sb = wp.tile([128, 2], dt.float32)
        nc.sync.dma_start(fcb_sb[:], t["fcb"].rearrange("(m p) o -> p (m o)", p=128))
        embT = [pp.tile([128, NL], dt.float32, name=f"embT{m}") for m in range(2)]
        emb_sb = pp.tile([NL, 256], dt.float32)
        for m in range(2):
            ps = psB.tile([128, NL], dt.float32, tag="gps")
            nc.tensor.matmul(ps[:], fcw_sb[:, m * 128:(m + 1) * 128], gapT[:],
                             start=True, stop=True)
            nc.scalar.activation(embT[m][:], ps[:], AF.Identity,
                                 bias=fcb_sb[:, m:m + 1])
            tp = psB.tile([128, 128], dt.float32, tag="gps")
            nc.tensor.transpose(tp[:], embT[m][:], ident_sb[:])
            nc.scalar.activation(emb_sb[:, m * 128:(m + 1) * 128], tp[:], AF.Copy)
        nc.sync.dma_start(t["emb_out"][:], emb_sb[:])

        # ---- AllGather embT ----
        ag_in = dramp.tile([256, NL], dt.float32)
        ag_out = dramp.tile([256 * N_CORES, NL], dt.float32, addr_space="Shared")
        for m in range(2):
            nc.sync.dma_start(ag_in[m * 128:(m + 1) * 128, :], embT[m][:])
        nc.gpsimd.collective_compute(
            "AllGather", ALU.bypass,
            replica_groups=[list(range(N_CORES))],
            ins=[ag_in[:]], outs=[ag_out[:]],
        )

        with (
            tc.tile_pool(name="sp", bufs=1) as sp,
            tc.tile_pool(name="g_evac", bufs=3) as ep,
        ):
            # embT_all as [128, 16*128]; free blocks b = 2r+k
            ebT = sp.tile([128, 16 * 128], dt.float32)
            nc.sync.dma_start(ebT[:], ag_out.rearrange("(b p) c -> p b c", p=128))
            ebTv = ebT.rearrange("p (r k c) -> p r k c", r=8, k=2, c=128)

            # NOTE: d2 is computed UNCENTERED, mirroring the reference's
            # sq_i + sq_j - 2*dot fp32 structure. The reference's own rounding
            # (cancellation at ~0.46 magnitude) decides several near-tie kNN
            # boundaries; a *more accurate* computation mismatches it. Matching
            # the structure makes our rounding correlate with the reference's.
            esq = sp.tile([128, 16 * 128], dt.float32)
            nc.scalar.activation(esq[:], ebT[:], AF.Square)
            esqv = esq.rearrange("p (r k c) -> p r k c", r=8, k=2, c=128)

            # local sq row via ones-lhsT matmul over squared embT
            sql_ps = psB.tile([1, 128], dt.float32, tag="gps")
            for k in range(2):
                lsq = ep.tile([128, 128], dt.float32, tag="lsq")
                nc.scalar.activation(lsq[:], embT[k][:], AF.Square)
                nc.tensor.matmul(sql_ps[:], ones_col[:], lsq[:],
                                 start=(k == 0), stop=(k == 1))
            sql_sb = sp.tile([1, 128], dt.float32)
            nc.scalar.activation(sql_sb[:], sql_ps[:], AF.Copy)

            # sq_all row + ones row (all partition-0: engine APs need
            # 32-aligned base partitions, so no [2, N] stacked tiles)
            ones_N = sp.tile([1, N], dt.float32)
            nc.vector.memset(ones_N[:], 1.0)
            sq_row = sp.tile([1, N], dt.float32)
            for half in range(2):
                sq_ps = psB.tile([1, 512], dt.float32, tag="gps")
                for k in range(2):
                    nc.tensor.matmul(sq_ps[:], ones_col[:],
                                     esqv[:, 4 * half:4 * half + 4, k, :],
                                     start=(k == 0), stop=(k == 1))
                nc.scalar.activation(sq_row[0:1, half * 512:(half + 1) * 512],
                                     sq_ps[:], AF.Copy)

            br_sb = ep.tile([1, N], dt.float32, tag="brsb")
            nc.sync.dma_start(br_sb[:], t["biasrow"][:])

            # ---- d2 = (bias_j + sq_j + sq_i) - 2*G, dist, selection ----
            # (dist output carries the <=6e-8 tie-bias; far below the
            # reference's own ~7.7e-4 rounding envelope on dist.)
            dist_sb = sp.tile([NL, N], dt.float32)
            adj_sb = sp.tile([NL, N], dt.float32)
            negd = sp.tile([NL, N], dt.float32)
            for half in range(2):
                cs = slice(half * 512, (half + 1) * 512)
                psG = psB.tile([128, 512], dt.float32, tag="gps")
                for k in range(2):
                    nc.tensor.matmul(psG[:], embT[k][:],
                                     ebTv[:, 4 * half:4 * half + 4, k, :],
                                     start=(k == 0), stop=(k == 1))
                g2 = ep.tile([128, 512], dt.float32, tag="g2")
                nc.scalar.activation(g2[:], psG[:], AF.Copy, scale=2.0)
                psT = psB.tile([128, 512], dt.float32, tag="gps")
                nc.tensor.matmul(psT[:], ones_row[:], br_sb[0:1, cs],
                                 start=True, stop=False, skip_group_check=True)
                nc.tensor.matmul(psT[:], ones_row[:], sq_row[0:1, cs],
                                 start=False, stop=False, skip_group_check=True)
                nc.tensor.matmul(psT[:], sql_sb[:], ones_N[0:1, cs],
                                 start=False, stop=True, skip_group_check=True)
                t_sb = ep.tile([128, 512], dt.float32, tag="t_sb")
                nc.scalar.activation(t_sb[:], psT[:], AF.Copy)
                d2b = ep.tile([128, 512], dt.float32, tag="d2b")
                nc.vector.tensor_sub(d2b[:], t_sb[:], g2[:])
                nc.vector.tensor_sub(negd[:, cs], g2[:], t_sb[:])
                rl = ep.tile([128, 512], dt.float32, tag="rl")
                nc.vector.tensor_scalar_max(rl[:], d2b[:], 0.0)
                nc.scalar.activation(dist_sb[:, cs], rl[:], AF.Sqrt)
            nc.sync.dma_start(t["dist_out"][:], dist_sb[:])

            m8 = ep.tile([128, 8], dt.float32, tag="m8")
            nc.vector.max(m8[:], negd[:])
            nc.vector.tensor_scalar(adj_sb[:], negd[:], m8[:, 5:6], None,
                                    op0=ALU.is_ge)
            nc.sync.dma_start(t["adj_out"][:], adj_sb[:])

            # ---- adjT blocks via PE transpose ----
            adjT = [sp.tile([128, 128], dt.float32, name=f"adjT{r}")
                    for r in range(8)]
            for r in range(8):
                tp = psB.tile([128, 128], dt.float32, tag="gps")
                nc.tensor.transpose(tp[:], adj_sb[:, r * 128:(r + 1) * 128],
                                    ident_sb[:])
                nc.scalar.activation(adjT[r][:], tp[:], AF.Copy)

            # ---- GCN weights ----
            gw1_sb = [wp.tile([128, 256], dt.float32, name=f"gw1_sb{k}")
                      for k in range(2)]
            gw2_sb = [wp.tile([128, 256], dt.float32, name=f"gw2_sb{k}")
                      for k in range(2)]
            for k in range(2):
                nc.sync.dma_start(gw1_sb[k][:], t["gw1"][k * 128:(k + 1) * 128, :])
                nc.sync.dma_start(gw2_sb[k][:], t["gw2"][k * 128:(k + 1) * 128, :])
            gb1_sb = wp.tile([128, 2], dt.float32)
            nc.sync.dma_start(gb1_sb[:],
                              t["gb1"].rearrange("(m p) o -> p (m o)", p=128))
            gb2_sb = wp.tile([128, 2], dt.float32)
            nc.sync.dma_start(gb2_sb[:],
                              t["gb2"].rearrange("(m p) o -> p (m o)", p=128))

            # layer 1: hg1[r] = emb_all[block r] @ gw1 (node-major out)
            hg1 = [sp.tile([128, 256], dt.float32, name=f"hg1_{r}")
                   for r in range(8)]
            for r in range(8):
                ps = psB.tile([128, 256], dt.float32, tag="gps")
                for k in range(2):
                    nc.tensor.matmul(ps[:], ebTv[:, r, k, :], gw1_sb[k][:],
                                     start=(k == 0), stop=(k == 1))
                nc.scalar.activation(hg1[r][:], ps[:], AF.Copy)
            x1T = [sp.tile([128, 128], dt.float32, name=f"x1T{m}")
                   for m in range(2)]
            for m in range(2):
                ps = psB.tile([128, 128], dt.float32, tag="gps")
                for r in range(8):
                    nc.tensor.matmul(ps[:], hg1[r][:, m * 128:(m + 1) * 128],
                                     adjT[r][:], start=(r == 0), stop=(r == 7))
                nc.scalar.activation(x1T[m][:], ps[:], AF.Relu,
                                     bias=gb1_sb[:, m:m + 1])

            # AllGather x1T
            ag2_in = dramp.tile([256, NL], dt.float32)
            ag2_out = dramp.tile([256 * N_CORES, NL], dt.float32,
                                 addr_space="Shared")
            for m in range(2):
                nc.sync.dma_start(ag2_in[m * 128:(m + 1) * 128, :], x1T[m][:])
            nc.gpsimd.collective_compute(
                "AllGather", ALU.bypass,
                replica_groups=[list(range(N_CORES))],
                ins=[ag2_in[:]], outs=[ag2_out[:]],
            )
            x1a = sp.tile([128, 16 * 128], dt.float32)
            nc.sync.dma_start(x1a[:], ag2_out.rearrange("(b p) c -> p b c", p=128))
            x1av = x1a.rearrange("p (r k c) -> p r k c", r=8, k=2, c=128)

            # layer 2
            hg2 = [sp.tile([128, 256], dt.float32, name=f"hg2_{r}")
                   for r in range(8)]
            for r in range(8):
                ps = psB.tile([128, 256], dt.float32, tag="gps")
                for k in range(2):
                    nc.tensor.matmul(ps[:], x1av[:, r, k, :], gw2_sb[k][:],
                                     start=(k == 0), stop=(k == 1))
                nc.scalar.activation(hg2[r][:], ps[:], AF.Copy)
            x2T = [sp.tile([128, 128], dt.float32, name=f"x2T{m}")
                   for m in range(2)]
            for m in range(2):
                ps = psB.tile([128, 128], dt.float32, tag="gps")
                for r in range(8):
                    nc.tensor.matmul(ps[:], hg2[r][:, m * 128:(m + 1) * 128],
                                     adjT[r][:], start=(r == 0), stop=(r == 7))
                nc.scalar.activation(x2T[m][:], ps[:], AF.Identity,
                                     bias=gb2_sb[:, m:m + 1])

            # ---- heads ----
            cw_sb = [wp.tile([128, 11], dt.float32, name=f"cw_sb{k}")
                     for k in range(2)]
            for k in range(2):
                nc.sync.dma_start(cw_sb[k][:], t["cwt"][k * 128:(k + 1) * 128, :])
            cb_sb = wp.tile([1, 11], dt.float32)
            nc.sync.dma_start(cb_sb[:], t["cbt"][:])
            ps = psB.tile([128, 11], dt.float32, tag="gps")
            for k in range(2):
                nc.tensor.matmul(ps[:], x2T[k][:], cw_sb[k][:], start=(k == 0),
                                 stop=False, skip_group_check=True)
            nc.tensor.matmul(ps[:], ones_row[:], cb_sb[:], start=False, stop=True,
                             skip_group_check=True)
            logit_sb = ep.tile([128, 11], dt.float32, tag="logit")
            nc.scalar.activation(logit_sb[:], ps[:], AF.Copy)
            nc.sync.dma_start(t["cls_out"][:], logit_sb[:, 0:10])
            nc.sync.dma_start(t["spr_out"][:], logit_sb[:, 10:11])


# ---------------------------------------------------------------------------
# host side
# ---------------------------------------------------------------------------

_NC_CACHE = {}


def _get_nc():
    if "nc" not in _NC_CACHE:
        _NC_CACHE["nc"] = build_nc()
    return _NC_CACHE["nc"]


def _f32(x):
    return np.ascontiguousarray(x, dtype=np.float32)


def prep_in_maps(images, w1, b1, w2, b2, aspp_w, aspp_b, proj_w, proj_b,
                 fc_w, fc_b, gcn1_w, gcn1_b, gcn2_w, gcn2_b, cls_w, cls_b,
                 spr_w, spr_b):
    # conv1 im2col on host: [27(ky,kx,c), 1024, 256]
    xp = np.pad(_f32(images), ((0, 0), (0, 0), (1, 1), (1, 1)))
    s = xp.strides
    v = np.lib.stride_tricks.as_strided(
        xp, (N, 3, 3, 3, 16, 16), (s[0], s[1], s[2], s[3], 2 * s[2], 2 * s[3]))
    col = _f32(v.transpose(2, 3, 1, 0, 4, 5).reshape(27, N, 256))

    vsr = np.float32(1.0) / np.sqrt(np.float32(6.0))
    c6 = np.float32(vsr * vsr)

    shared = dict(
        w1c=_f32(np.asarray(w1).transpose(2, 3, 1, 0).reshape(27, 64)),
        b1c=_f32(b1).reshape(64, 1),
        w2t=_f32(np.asarray(w2).transpose(2, 3, 1, 0).reshape(9, 64, 128)),
        b2c=_f32(b2).reshape(128, 1),
        awt=_f32(np.asarray(aspp_w).transpose(0, 3, 4, 2, 1).reshape(4, 9, 128, 128)),
        abt=_f32(aspp_b).reshape(4, 128, 1),
        pwt=_f32(np.asarray(proj_w)[:, :, 0, 0].T),
        pbt=_f32(proj_b).reshape(128, 1),
        fcw=_f32(fc_w),
        fcb=_f32(fc_b).reshape(256, 1),
        gw1=_f32(c6 * np.asarray(gcn1_w, dtype=np.float32)),
        gb1=_f32(gcn1_b).reshape(256, 1),
        gw2=_f32(c6 * np.asarray(gcn2_w, dtype=np.float32)),
        gb2=_f32(gcn2_b).reshape(256, 1),
        cwt=_f32(np.concatenate([np.asarray(cls_w), np.asarray(spr_w)], axis=1)),
        cbt=_f32(np.concatenate([np.asarray(cls_b), np.asarray(spr_b)]))[None, :],
        biasrow=_f32(np.arange(N) * EPS_BIAS)[None, :],
        ident=np.eye(128, dtype=np.float32),
    )
    in_maps = []
    for c in range(N_CORES):
        m = dict(shared)
        m["im2col1"] = _f32(col[:, c * NL:(c + 1) * NL, :].reshape(27, NL * 256))
        in_maps.append(m)
    return in_maps


def run_on_device(in_maps):
    nc = _get_nc()
    return run_bass_kernel_spmd(nc, in_maps, core_ids=list(range(N_CORES)))


def kernel(**inputs):
    in_maps = prep_in_maps(**inputs)
    res = run_on_device(in_maps)
    emb = np.concatenate([res.results[c]["emb_out"] for c in range(N_CORES)], 0)
    adj = np.concatenate([res.results[c]["adj_out"] for c in range(N_CORES)], 0)
    dist = np.concatenate([res.results[c]["dist_out"] for c in range(N_CORES)], 0)
    lc = np.concatenate([res.results[c]["cls_out"] for c in range(N_CORES)], 0)
    ls = np.concatenate([res.results[c]["spr_out"][:, 0] for c in range(N_CORES)], 0)
    return emb, adj, dist, lc, ls
